# revision 19
# baseline (speedup 1.0000x reference)
"""Fused sparse-attention kernel for TRN2, SPMD over 8 NeuronCores.

Sharding: data-parallel over batch (32 -> 4 per core). Per core, the full
block (LayerNorm -> fused qkv -> per-head attention with gathered relative
position bias -> proj) is computed on-chip; attention probabilities never
touch HBM.

v2 restructure vs v1:
- All q/k/v/proj biases handled EXACTLY at ~zero on-chip cost:
  * q-bias bq enters scores only through beta[m] = scale*bq.k[m] (the
    query-side terms are constant over the softmax axis and cancel); beta
    is produced by one tiny extra matmul (w_beta = scale*Wk^T bq) and folded
    into the per-partition bias operand of the exp ACTIVATE.
  * v-bias: p@(v+bv) = p@v + sums*bv, and the normalize divides by sums, so
    bv contributes exactly bv per head dim -> folded into proj bias on host
    (bp_eff = bp + Wp @ bv).
- Softmax denominators: 1/sums via reciprocal_approx_fast (single custom-DVE
  op) instead of the 4-8us iterative-divide reciprocal per row.
- Normalize: GPSIMD partition_broadcast of the reciprocal row, then one f16
  DVE multiply into ot (PSUM drained by a single DVE copy so o-accumulator
  banks recycle fast).
- PSUM: s tiles [128,1024] f32 (2 banks, bufs=2) + o accumulators
  [65,1024] f32 x2 = exactly 8 banks; S matmuls of the next tile overlap the
  exp of the current one.
- E-multiply (ps *= exp(bias-table)) split between DVE and GPSIMD to
  balance engine load (GP_MOD knob).
"""

import os
import sys

import numpy as np

for _p in ("/opt/trn_rl_repo", "/root/.axon_site/_ro/trn_rl_repo"):
    if os.path.isdir(_p) and _p not in sys.path:
        sys.path.insert(0, _p)

import concourse.bacc as bacc
import concourse.tile as tile
from concourse import bass_utils, mybir
from concourse.masks import make_identity

F32 = mybir.dt.float32
F16 = mybir.dt.float16

NCORES = 8
B_TOTAL = 32
NB = B_TOTAL // NCORES  # local batch per core
N = 1024
NT = 8        # 128-row tiles over n
DIM = 256
CC = 2        # 128-row chunks over DIM
H = 8
KD = 16
D = 64
MC = 8        # 128-row chunks over m
EPS = 1e-5
OFF = float(4.0 * np.log(2.0))  # exp offset for fp16 headroom (cancels)

# E-multiply engine split: tile idx t (0..255); t % GP_MOD < GP_CNT -> GPSIMD
GP_MOD = 10
GP_CNT = 0


def _emit(tc, aps):
    nc = tc.nc
    x, wqk, wv, wp, wbeta, bp, etab, out = aps

    with tc.tile_pool(name="persist", bufs=1) as persist:
        # --- constants / weights resident in SBUF ---
        wqk_sb = persist.tile([128, CC, 4, 128], F16)
        nc.sync.dma_start(out=wqk_sb, in_=wqk.rearrange("cc ci jt j -> ci cc jt j"))
        wv_sb = persist.tile([128, CC, 512], F16)
        nc.sync.dma_start(out=wv_sb, in_=wv.rearrange("cc ci v -> ci cc v"))
        wp_sb = persist.tile([128, 4, 256], F16)
        nc.sync.dma_start(out=wp_sb, in_=wp.rearrange("cc ci c -> ci cc c"))
        wbeta_sb = persist.tile([128, CC, H], F16)
        nc.sync.dma_start(out=wbeta_sb, in_=wbeta.rearrange("cc ci h -> ci cc h"))
        bp_sb = persist.tile([128, 512], F32)
        nc.sync.dma_start(out=bp_sb, in_=bp.partition_broadcast(128))
        ident = persist.tile([128, 128], F16)
        make_identity(nc, ident)
        epsv = persist.tile([128, 1], F32)
        nc.vector.memset(epsv, EPS)

        qkT_l = []  # per-b [128, 4, 1024] f16: jt tiles (kT g0, qT g0, kT g1, qT g1)
        v_l = []    # per-b [128, NT, H, 65] f16: V rows + ones column per head
        ot_l = []   # per-b [128, 4, 1024] f16: O.T (dh on partitions, 4 chunks)
        boff_l = []  # per-b [128, MC, H] f32: beta[m,h] - OFF (exp bias operand)

        # ---------------- phase 1: LN, xn.T, qkv projections ----------------
        with (
            tc.tile_pool(name="p1", bufs=2) as p1,
            tc.tile_pool(name="p1ps", bufs=2, space="PSUM") as p1ps,
        ):
            for b in range(NB):
                x_sb = p1.tile([128, NT, DIM], F32, tag="x", bufs=2)
                nc.sync.dma_start(
                    out=x_sb, in_=x[b].rearrange("(t p) c -> p t c", p=128)
                )
                xn_sb = p1.tile([128, NT, DIM], F16, tag="xn", bufs=2)
                for t in range(NT):
                    stats = p1.tile([128, 6], F32, tag="stats", bufs=3)
                    nc.vector.bn_stats(out=stats, in_=x_sb[:, t])
                    mv = p1.tile([128, 2], F32, tag="mv", bufs=3)
                    nc.vector.bn_aggr(out=mv, in_=stats)
                    rstd = p1.tile([128, 1], F32, tag="rstd", bufs=3)
                    nc.scalar.activation(
                        out=rstd, in_=mv[:, 1:2],
                        func=mybir.ActivationFunctionType.Sqrt,
                        bias=epsv, scale=1.0,
                    )
                    nc.vector.reciprocal(out=rstd, in_=rstd)
                    nc.vector.tensor_scalar(
                        out=xn_sb[:, t], in0=x_sb[:, t],
                        scalar1=mv[:, 0:1], scalar2=rstd,
                        op0=mybir.AluOpType.subtract, op1=mybir.AluOpType.mult,
                    )
                # xn.T via PE transpose (copies batched per 2 tiles)
                xnT = p1.tile([128, CC, N], F16, tag="xnt", bufs=2)
                for cc in range(CC):
                    for t2 in range(NT // 2):
                        tp = p1ps.tile([128, 2, 128], F16, tag="tp", bufs=2)
                        for i in range(2):
                            t = 2 * t2 + i
                            nc.tensor.transpose(
                                tp[:, i], xn_sb[:, t, cc * 128:(cc + 1) * 128],
                                ident,
                            )
                        nc.scalar.copy(
                            out=xnT[:, cc, t2 * 256:(t2 + 1) * 256],
                            in_=tp,
                        )
                # q.T / k.T, packed by 32-row strips per head (zeros padding)
                qkT = persist.tile([128, 4, N], F16, tag="qkT", bufs=NB, name="qkT")
                for jt in range(4):
                    qkp = p1ps.tile([128, N], F32, tag="qkp", bufs=1)
                    for nh in range(2):
                        for cc in range(CC):
                            nc.tensor.matmul(
                                qkp[:, nh * 512:(nh + 1) * 512],
                                lhsT=wqk_sb[:, cc, jt],
                                rhs=xnT[:, cc, nh * 512:(nh + 1) * 512],
                                start=(cc == 0), stop=(cc == CC - 1),
                            )
                    nc.scalar.copy(out=qkT[:, jt], in_=qkp)
                qkT_l.append(qkT)
                # beta[m, h] = scale * bq_h . k(m)  (exp bias; exact bq fold)
                boff = persist.tile([128, MC, H], F32, tag="boff", bufs=NB,
                                    name="boff")
                bps = p1ps.tile([128, NT, H], F32, tag="bps", bufs=1)
                for t in range(NT):
                    for cc in range(CC):
                        nc.tensor.matmul(
                            bps[:, t],
                            lhsT=xnT[:, cc, t * 128:(t + 1) * 128],
                            rhs=wbeta_sb[:, cc],
                            start=(cc == 0), stop=(cc == CC - 1),
                        )
                nc.vector.tensor_scalar(
                    out=boff, in0=bps,
                    scalar1=-OFF, scalar2=None,
                    op0=mybir.AluOpType.add,
                )
                boff_l.append(boff)
                # V rows (no bias; bv folded into bp_eff) + ones column
                v_sb = persist.tile([128, NT, H, 65], F16, tag="v", bufs=NB,
                                    name="v_sb")
                nc.vector.memset(v_sb[:, :, :, 64:65], 1.0)
                for t2 in range(NT // 2):
                    vp = p1ps.tile([128, 2, 512], F32, tag="vp", bufs=1)
                    for i in range(2):
                        t = 2 * t2 + i
                        for cc in range(CC):
                            nc.tensor.matmul(
                                vp[:, i],
                                lhsT=xnT[:, cc, t * 128:(t + 1) * 128],
                                rhs=wv_sb[:, cc],
                                start=(cc == 0), stop=(cc == CC - 1),
                            )
                    nc.vector.tensor_copy(
                        out=v_sb[:, 2 * t2:2 * t2 + 2, :, 0:64],
                        in_=vp.rearrange("p i (h d) -> p i h d", d=64),
                    )
                v_l.append(v_sb)
                ot = persist.tile([128, 4, N], F16, tag="ot", bufs=NB, name="ot")
                ot_l.append(ot)

        # ---------------- phase 2: attention per head pair ----------------
        tile_idx = 0
        with (
            tc.tile_pool(name="p2", bufs=2) as p2,
            tc.tile_pool(name="p2ps", bufs=2, space="PSUM") as p2ps,
        ):
            for g in range(4):  # head pair {2g, 2g+1}
                e_sb = p2.tile([128, MC, 2, N], F16, tag="etab", bufs=2)
                nc.sync.dma_start(out=e_sb, in_=etab[g].rearrange(
                    "mc hp p n -> p mc hp n"))
                for b in range(NB):
                    # O'.T accumulators: [65, n] = V'.T @ P.T; row 64 = sums
                    o_ts = [
                        p2ps.tile([65, N], F32, tag="o", bufs=2, name="o_ts")
                        for _ in range(2)
                    ]

                    def emit_pv(mc, ps_pair):
                        # PV: V' (with ones column) stationary, P.T streams;
                        # lagged one mc step behind S so the PE queue always
                        # has ready S matmuls ahead of the exp-dependent PVs
                        for nh in range(2):
                            for hp in range(2):
                                nc.tensor.matmul(
                                    o_ts[hp][:, nh * 512:(nh + 1) * 512],
                                    lhsT=v_l[b][:, mc, 2 * g + hp],
                                    rhs=ps_pair[:, hp, nh * 512:(nh + 1) * 512],
                                    start=(mc == 0), stop=(mc == MC - 1),
                                    skip_group_check=True,
                                )

                    prev = None  # (mc, ps_pair)
                    for mc in range(MC):
                        s_tiles = [
                            p2ps.tile([128, N], F32, tag="s", bufs=2, name="s_ps")
                            for _ in range(2)
                        ]
                        # S matmuls, strip-alternated so the two heads' row
                        # tiles execute concurrently in the PE array
                        for nh in range(2):
                            for hp in range(2):
                                h = 2 * g + hp
                                jt = 2 * (h // 4)
                                strip = 32 * (h % 4)
                                nc.tensor.matmul(
                                    s_tiles[hp][:, nh * 512:(nh + 1) * 512],
                                    lhsT=qkT_l[b][strip:strip + KD, jt,
                                                  mc * 128:(mc + 1) * 128],
                                    rhs=qkT_l[b][strip:strip + KD, jt + 1,
                                                 nh * 512:(nh + 1) * 512],
                                    start=True, stop=True,
                                    tile_position=(strip, 0),
                                )
                        ps_pair = p2.tile([128, 2, N], F16, tag="ps", bufs=3,
                                          name="ps")
                        for hp in range(2):
                            h = 2 * g + hp
                            nc.scalar.activation(
                                out=ps_pair[:, hp], in_=s_tiles[hp],
                                func=mybir.ActivationFunctionType.Exp,
                                bias=boff_l[b][:, mc, h:h + 1], scale=1.0,
                            )
                        nc.vector.tensor_tensor(
                            out=ps_pair, in0=ps_pair, in1=e_sb[:, mc],
                            op=mybir.AluOpType.mult,
                        )
                        if prev is not None:
                            emit_pv(*prev)
                        prev = (mc, ps_pair)
                    emit_pv(*prev)
                    # tail: recip of sums, drain, broadcast, normalize
                    for hp in range(2):
                        srow = p2.tile([1, N], F32, tag="srow", bufs=1)
                        nc.vector.tensor_copy(out=srow, in_=o_ts[hp][64:65, :])
                        sr = p2.tile([1, N], F32, tag="sr", bufs=2)
                        nc.vector.reciprocal_approx_fast(out=sr, in_=srow)
                        raw = p2.tile([64, N], F16, tag="raw", bufs=2)
                        nc.vector.tensor_copy(out=raw, in_=o_ts[hp][0:64, :])
                        srf = p2.tile([1, N], F16, tag="srf", bufs=2)
                        nc.vector.tensor_copy(out=srf, in_=sr)
                        rb = p2.tile([64, N], F16, tag="rb", bufs=2)
                        nc.gpsimd.partition_broadcast(rb, srf)
                        nc.vector.tensor_tensor(
                            out=ot_l[b][64 * hp:64 * hp + 64, g, :],
                            in0=raw, in1=rb,
                            op=mybir.AluOpType.mult,
                        )

        # ---------------- phase 3: output projection ----------------
        with (
            tc.tile_pool(name="p3", bufs=2) as p3,
            tc.tile_pool(name="p3ps", bufs=4, space="PSUM") as p3ps,
        ):
            for b in range(NB):
                o_sb = p3.tile([128, NT, 256], F32, tag="osb", bufs=2)
                for n2 in range(NT // 2):
                    y = p3ps.tile([128, 2, 256], F32, tag="y", bufs=2)
                    for i in range(2):
                        nt = 2 * n2 + i
                        for cc2 in range(4):
                            nc.tensor.matmul(
                                y[:, i],
                                lhsT=ot_l[b][:, cc2, nt * 128:(nt + 1) * 128],
                                rhs=wp_sb[:, cc2],
                                start=(cc2 == 0), stop=(cc2 == 3),
                            )
                    nc.vector.tensor_tensor(
                        out=o_sb[:, 2 * n2:2 * n2 + 2],
                        in0=y,
                        in1=bp_sb.rearrange("p (i c) -> p i c", i=2),
                        op=mybir.AluOpType.add,
                    )
                nc.sync.dma_start(
                    out=out[b].rearrange("(t p) c -> p t c", p=128), in_=o_sb
                )


def build_module():
    nc = bacc.Bacc(
        "TRN2",
        target_bir_lowering=False,
        debug=False,
        enable_asserts=False,
        num_devices=NCORES,
    )
    x_t = nc.dram_tensor("x", [NB, N, DIM], F32, kind="ExternalInput")
    wqk_t = nc.dram_tensor("wqk", [CC, 128, 4, 128], F16, kind="ExternalInput")
    wv_t = nc.dram_tensor("wv", [CC, 128, 512], F16, kind="ExternalInput")
    wp_t = nc.dram_tensor("wp", [4, 128, 256], F16, kind="ExternalInput")
    wbeta_t = nc.dram_tensor("wbeta", [CC, 128, H], F16, kind="ExternalInput")
    bp_t = nc.dram_tensor("bp", [512], F32, kind="ExternalInput")
    e_t = nc.dram_tensor("etab", [4, MC, 2, 128, N], F16, kind="ExternalInput")
    out_t = nc.dram_tensor("out", [NB, N, DIM], F32, kind="ExternalOutput")

    aps = [t.ap() for t in (x_t, wqk_t, wv_t, wp_t, wbeta_t, bp_t, e_t, out_t)]
    with tile.TileContext(nc) as tc:
        _emit(tc, aps)
    nc.compile()
    return nc


def prep_inputs(inputs):
    """Host-side prep: fold norm affine + scale into weights, pack q/k rows
    into 32-row strips for PE row-tiling, fold bv into the proj bias, build
    the beta weight (exact bq fold) and materialize E = exp(bias)."""
    x = np.asarray(inputs["x"], np.float32)
    norm_w = np.asarray(inputs["norm_w"], np.float32)
    norm_b = np.asarray(inputs["norm_b"], np.float32)
    qkv_w = np.asarray(inputs["qkv_w"], np.float32)
    qkv_b = np.asarray(inputs["qkv_b"], np.float32)
    proj_w = np.asarray(inputs["proj_w"], np.float32)
    proj_b = np.asarray(inputs["proj_b"], np.float32)
    ab = np.asarray(inputs["attn_biases"], np.float32)
    bi = np.asarray(inputs["bias_idxs"], np.int64)

    scale = KD ** -0.5
    wr = qkv_w.reshape(H, 2 * KD + D, DIM)
    br = qkv_b.reshape(H, 2 * KD + D)
    # fold norm_w into weights, norm_b into biases
    w_eff = wr * norm_w[None, None, :]
    b_eff = br + wr @ norm_b
    w_q = w_eff[:, :KD] * scale
    b_q = b_eff[:, :KD]          # enters via beta only (softmax-exact fold)
    w_k = w_eff[:, KD:2 * KD]
    w_v = w_eff[:, 2 * KD:]
    b_v = b_eff[:, 2 * KD:]

    wqk = np.zeros((CC, 128, 4, 128), np.float16)
    for jt in range(4):
        kind_q = jt % 2 == 1
        hg = jt // 2
        w_src = w_q if kind_q else w_k
        for hp in range(4):
            h = hg * 4 + hp
            w_jc = w_src[h]  # [KD, DIM]
            for cc in range(CC):
                wqk[cc, :, jt, 32 * hp:32 * hp + KD] = (
                    w_jc[:, cc * 128:(cc + 1) * 128].T.astype(np.float16)
                )

    # w_beta[:, h] = scale * Wk_h^T @ bq_h  ->  beta[m,h] = scale*bq.k0(m)
    wbeta_full = np.zeros((DIM, H), np.float32)
    for h in range(H):
        wbeta_full[:, h] = scale * (w_k[h].T @ b_q[h])
    wbeta = np.zeros((CC, 128, H), np.float16)
    for cc in range(CC):
        wbeta[cc] = wbeta_full[cc * 128:(cc + 1) * 128].astype(np.float16)

    wv = np.zeros((CC, 128, 512), np.float16)
    for cc in range(CC):
        wv[cc] = w_v.reshape(512, DIM)[:, cc * 128:(cc + 1) * 128].T.astype(np.float16)

    wp = np.zeros((4, 128, 256), np.float16)
    for cc2 in range(4):
        wp[cc2] = proj_w[:, cc2 * 128:(cc2 + 1) * 128].T.astype(np.float16)
    # bv folded: out = (o/s + bv) @ Wp^T + bp = (o/s) @ Wp^T + (bp + Wp@bv)
    bp = np.tile((proj_b + proj_w @ b_v.reshape(512)).astype(np.float32), 2)

    etab = np.exp(ab[:, bi]).astype(np.float16)          # [H, 1024, 1024]
    etab = etab.reshape(4, 2, MC, 128, N).transpose(0, 2, 1, 3, 4).copy()

    shared = {
        "wqk": wqk, "wv": wv, "wp": wp, "wbeta": wbeta, "bp": bp, "etab": etab,
    }
    in_maps = []
    for c in range(NCORES):
        m = dict(shared)
        m["x"] = np.ascontiguousarray(x[c * NB:(c + 1) * NB])
        in_maps.append(m)
    return in_maps


_NC_CACHE = None


def _get_nc():
    global _NC_CACHE
    if _NC_CACHE is None:
        _NC_CACHE = build_module()
    return _NC_CACHE


def run(inputs, **spmd_kwargs):
    nc = _get_nc()
    in_maps = prep_inputs(inputs)
    res = bass_utils.run_bass_kernel_spmd(
        nc, in_maps, core_ids=list(range(NCORES)), **spmd_kwargs
    )
    out = np.concatenate([res.results[c]["out"] for c in range(NCORES)], axis=0)
    return out.astype(np.float32), res


def kernel(**inputs):
    out, _ = run(inputs)
    return out


if __name__ == "__main__":
    print("building module...")
    nc = _get_nc()
    print("module built ok")


# revision 20
# speedup vs baseline: 6537.1576x; 6537.1576x over previous
"""Fused sparse-attention kernel for TRN2, SPMD over 8 NeuronCores.

Sharding: data-parallel over batch (32 -> 4 per core). Per core, the full
block (LayerNorm -> fused qkv -> per-head attention with gathered relative
position bias -> proj) is computed on-chip; attention probabilities never
touch HBM.

v2 restructure vs v1:
- All q/k/v/proj biases handled EXACTLY at ~zero on-chip cost:
  * q-bias bq enters scores only through beta[m] = scale*bq.k[m] (the
    query-side terms are constant over the softmax axis and cancel); beta
    is produced by one tiny extra matmul (w_beta = scale*Wk^T bq) and folded
    into the per-partition bias operand of the exp ACTIVATE.
  * v-bias: p@(v+bv) = p@v + sums*bv, and the normalize divides by sums, so
    bv contributes exactly bv per head dim -> folded into proj bias on host
    (bp_eff = bp + Wp @ bv).
- Softmax denominators: 1/sums via reciprocal_approx_fast (single custom-DVE
  op) instead of the 4-8us iterative-divide reciprocal per row.
- Normalize: GPSIMD partition_broadcast of the reciprocal row, then one f16
  DVE multiply into ot (PSUM drained by a single DVE copy so o-accumulator
  banks recycle fast).
- PSUM: s tiles [128,1024] f32 (2 banks, bufs=2) + o accumulators
  [65,1024] f32 x2 = exactly 8 banks; S matmuls of the next tile overlap the
  exp of the current one.
- E-multiply (ps *= exp(bias-table)) split between DVE and GPSIMD to
  balance engine load (GP_MOD knob).
"""

import os
import sys

import numpy as np

for _p in ("/opt/trn_rl_repo", "/root/.axon_site/_ro/trn_rl_repo"):
    if os.path.isdir(_p) and _p not in sys.path:
        sys.path.insert(0, _p)

import concourse.bacc as bacc
import concourse.tile as tile
from concourse import bass_utils, mybir
from concourse.masks import make_identity

F32 = mybir.dt.float32
F16 = mybir.dt.float16

NCORES = 8
B_TOTAL = 32
NB = B_TOTAL // NCORES  # local batch per core
N = 1024
NT = 8        # 128-row tiles over n
DIM = 256
CC = 2        # 128-row chunks over DIM
H = 8
KD = 16
D = 64
MC = 8        # 128-row chunks over m
EPS = 1e-5
OFF = float(4.0 * np.log(2.0))  # exp offset for fp16 headroom (cancels)

# E-multiply engine split: tile idx t (0..255); t % GP_MOD < GP_CNT -> GPSIMD
GP_MOD = 10
GP_CNT = 0


def _emit(tc, aps):
    nc = tc.nc
    x, wqk, wv, wp, wbeta, bp, etab, out = aps

    with tc.tile_pool(name="persist", bufs=1) as persist:
        # --- constants / weights resident in SBUF ---
        wqk_sb = persist.tile([128, CC, 4, 128], F16)
        nc.sync.dma_start(out=wqk_sb, in_=wqk.rearrange("cc ci jt j -> ci cc jt j"))
        wv_sb = persist.tile([128, CC, 512], F16)
        nc.sync.dma_start(out=wv_sb, in_=wv.rearrange("cc ci v -> ci cc v"))
        wp_sb = persist.tile([128, 4, 256], F16)
        nc.sync.dma_start(out=wp_sb, in_=wp.rearrange("cc ci c -> ci cc c"))
        wbeta_sb = persist.tile([128, CC, H], F16)
        nc.sync.dma_start(out=wbeta_sb, in_=wbeta.rearrange("cc ci h -> ci cc h"))
        bp_sb = persist.tile([128, 512], F32)
        nc.sync.dma_start(out=bp_sb, in_=bp.partition_broadcast(128))
        ident = persist.tile([128, 128], F16)
        make_identity(nc, ident)
        epsv = persist.tile([128, 1], F32)
        nc.vector.memset(epsv, EPS)

        qkT_l = []  # per-b [128, 4, 1024] f16: jt tiles (kT g0, qT g0, kT g1, qT g1)
        v_l = []    # per-b [128, NT, H, 65] f16: V rows + ones column per head
        ot_l = []   # per-b [128, 4, 1024] f16: O.T (dh on partitions, 4 chunks)
        boff_l = []  # per-b [128, MC, H] f32: beta[m,h] - OFF (exp bias operand)

        # ---------------- phase 1: LN, xn.T, qkv projections ----------------
        with (
            tc.tile_pool(name="p1", bufs=2) as p1,
            tc.tile_pool(name="p1ps", bufs=2, space="PSUM") as p1ps,
        ):
            for b in range(NB):
                x_sb = p1.tile([128, NT, DIM], F32, tag="x", bufs=2)
                nc.sync.dma_start(
                    out=x_sb, in_=x[b].rearrange("(t p) c -> p t c", p=128)
                )
                xn_sb = p1.tile([128, NT, DIM], F16, tag="xn", bufs=2)
                for t in range(NT):
                    stats = p1.tile([128, 6], F32, tag="stats", bufs=3)
                    nc.vector.bn_stats(out=stats, in_=x_sb[:, t])
                    mv = p1.tile([128, 2], F32, tag="mv", bufs=3)
                    nc.vector.bn_aggr(out=mv, in_=stats)
                    rstd = p1.tile([128, 1], F32, tag="rstd", bufs=3)
                    nc.scalar.activation(
                        out=rstd, in_=mv[:, 1:2],
                        func=mybir.ActivationFunctionType.Sqrt,
                        bias=epsv, scale=1.0,
                    )
                    nc.vector.reciprocal(out=rstd, in_=rstd)
                    nc.vector.tensor_scalar(
                        out=xn_sb[:, t], in0=x_sb[:, t],
                        scalar1=mv[:, 0:1], scalar2=rstd,
                        op0=mybir.AluOpType.subtract, op1=mybir.AluOpType.mult,
                    )
                # xn.T via PE transpose (copies batched per 2 tiles)
                xnT = p1.tile([128, CC, N], F16, tag="xnt", bufs=2)
                for cc in range(CC):
                    for t2 in range(NT // 2):
                        tp = p1ps.tile([128, 2, 128], F16, tag="tp", bufs=2)
                        for i in range(2):
                            t = 2 * t2 + i
                            nc.tensor.transpose(
                                tp[:, i], xn_sb[:, t, cc * 128:(cc + 1) * 128],
                                ident,
                            )
                        nc.scalar.copy(
                            out=xnT[:, cc, t2 * 256:(t2 + 1) * 256],
                            in_=tp,
                        )
                # q.T / k.T, packed by 32-row strips per head (zeros padding)
                qkT = persist.tile([128, 4, N], F16, tag="qkT", bufs=NB, name="qkT")
                for jt in range(4):
                    qkp = p1ps.tile([128, N], F32, tag="qkp", bufs=1)
                    for nh in range(2):
                        for cc in range(CC):
                            nc.tensor.matmul(
                                qkp[:, nh * 512:(nh + 1) * 512],
                                lhsT=wqk_sb[:, cc, jt],
                                rhs=xnT[:, cc, nh * 512:(nh + 1) * 512],
                                start=(cc == 0), stop=(cc == CC - 1),
                            )
                    nc.scalar.copy(out=qkT[:, jt], in_=qkp)
                qkT_l.append(qkT)
                # beta[m, h] = scale * bq_h . k(m)  (exp bias; exact bq fold)
                boff = persist.tile([128, MC, H], F32, tag="boff", bufs=NB,
                                    name="boff")
                bps = p1ps.tile([128, NT, H], F32, tag="bps", bufs=1)
                for t in range(NT):
                    for cc in range(CC):
                        nc.tensor.matmul(
                            bps[:, t],
                            lhsT=xnT[:, cc, t * 128:(t + 1) * 128],
                            rhs=wbeta_sb[:, cc],
                            start=(cc == 0), stop=(cc == CC - 1),
                        )
                nc.vector.tensor_scalar(
                    out=boff, in0=bps,
                    scalar1=-OFF, scalar2=None,
                    op0=mybir.AluOpType.add,
                )
                boff_l.append(boff)
                # V rows (no bias; bv folded into bp_eff) + ones column
                v_sb = persist.tile([128, NT, H, 65], F16, tag="v", bufs=NB,
                                    name="v_sb")
                nc.vector.memset(v_sb[:, :, :, 64:65], 1.0)
                for t2 in range(NT // 2):
                    vp = p1ps.tile([128, 2, 512], F32, tag="vp", bufs=1)
                    for i in range(2):
                        t = 2 * t2 + i
                        for cc in range(CC):
                            nc.tensor.matmul(
                                vp[:, i],
                                lhsT=xnT[:, cc, t * 128:(t + 1) * 128],
                                rhs=wv_sb[:, cc],
                                start=(cc == 0), stop=(cc == CC - 1),
                            )
                    nc.vector.tensor_copy(
                        out=v_sb[:, 2 * t2:2 * t2 + 2, :, 0:64],
                        in_=vp.rearrange("p i (h d) -> p i h d", d=64),
                    )
                v_l.append(v_sb)
                ot = persist.tile([128, 4, N], F16, tag="ot", bufs=NB, name="ot")
                ot_l.append(ot)

        # ---------------- phase 2: attention per head pair ----------------
        tile_idx = 0
        with (
            tc.tile_pool(name="p2", bufs=2) as p2,
            tc.tile_pool(name="p2ps", bufs=2, space="PSUM") as p2ps,
        ):
            for g in range(4):  # head pair {2g, 2g+1}
                e_sb = p2.tile([128, MC, 2, N], F16, tag="etab", bufs=2)
                nc.sync.dma_start(out=e_sb, in_=etab[g].rearrange(
                    "mc hp p n -> p mc hp n"))
                for b in range(NB):
                    # O'.T accumulators: [65, n] = V'.T @ P.T; row 64 = sums
                    o_ts = [
                        p2ps.tile([65, N], F32, tag="o", bufs=2, name="o_ts")
                        for _ in range(2)
                    ]

                    def emit_pv(mc, ps_hp, hp):
                        # PV: V' (with ones column) stationary, P.T streams;
                        # lagged one mc step behind S so the PE queue always
                        # has ready S matmuls ahead of the exp-dependent PVs.
                        # hp-granular: PV(hp) only waits its own head's
                        # E-multiply; nh-inner keeps the same lhsT for both
                        # matmuls (one LDWEIGHTS serialization, not two).
                        for nh in range(2):
                            nc.tensor.matmul(
                                o_ts[hp][:, nh * 512:(nh + 1) * 512],
                                lhsT=v_l[b][:, mc, 2 * g + hp],
                                rhs=ps_hp[:, nh * 512:(nh + 1) * 512],
                                start=(mc == 0), stop=(mc == MC - 1),
                                skip_group_check=True,
                            )

                    lag = []  # [(mc, ps_hp, hp), ...] pending PV emissions
                    for mc in range(MC):
                        s_tiles = [
                            p2ps.tile([128, N], F32, tag="s", bufs=2, name="s_ps")
                            for _ in range(2)
                        ]
                        # S matmuls, strip-alternated so the two heads' row
                        # tiles execute concurrently in the PE array
                        for nh in range(2):
                            for hp in range(2):
                                h = 2 * g + hp
                                jt = 2 * (h // 4)
                                strip = 32 * (h % 4)
                                nc.tensor.matmul(
                                    s_tiles[hp][:, nh * 512:(nh + 1) * 512],
                                    lhsT=qkT_l[b][strip:strip + KD, jt,
                                                  mc * 128:(mc + 1) * 128],
                                    rhs=qkT_l[b][strip:strip + KD, jt + 1,
                                                 nh * 512:(nh + 1) * 512],
                                    start=True, stop=True,
                                    tile_position=(strip, 0),
                                )
                        for hp in range(2):
                            h = 2 * g + hp
                            ps = p2.tile([128, N], F16, tag="ps", bufs=6,
                                         name="ps")
                            nc.scalar.activation(
                                out=ps, in_=s_tiles[hp],
                                func=mybir.ActivationFunctionType.Exp,
                                bias=boff_l[b][:, mc, h:h + 1], scale=1.0,
                            )
                            nc.vector.tensor_tensor(
                                out=ps, in0=ps, in1=e_sb[:, mc, hp],
                                op=mybir.AluOpType.mult,
                            )
                            lag.append((mc, ps, hp))
                            if len(lag) > 2:
                                emit_pv(*lag.pop(0))
                    for args in lag:
                        emit_pv(*args)
                    # tail: recip of sums, drain, broadcast, normalize
                    for hp in range(2):
                        srow = p2.tile([1, N], F32, tag="srow", bufs=1)
                        nc.vector.tensor_copy(out=srow, in_=o_ts[hp][64:65, :])
                        sr = p2.tile([1, N], F32, tag="sr", bufs=2)
                        nc.vector.reciprocal_approx_fast(out=sr, in_=srow)
                        raw = p2.tile([64, N], F16, tag="raw", bufs=2)
                        nc.vector.tensor_copy(out=raw, in_=o_ts[hp][0:64, :])
                        srf = p2.tile([1, N], F16, tag="srf", bufs=2)
                        nc.vector.tensor_copy(out=srf, in_=sr)
                        rb = p2.tile([64, N], F16, tag="rb", bufs=2)
                        nc.gpsimd.partition_broadcast(rb, srf)
                        nc.vector.tensor_tensor(
                            out=ot_l[b][64 * hp:64 * hp + 64, g, :],
                            in0=raw, in1=rb,
                            op=mybir.AluOpType.mult,
                        )

        # ---------------- phase 3: output projection ----------------
        with (
            tc.tile_pool(name="p3", bufs=2) as p3,
            tc.tile_pool(name="p3ps", bufs=4, space="PSUM") as p3ps,
        ):
            for b in range(NB):
                o_sb = p3.tile([128, NT, 256], F32, tag="osb", bufs=2)
                for n2 in range(NT // 2):
                    y = p3ps.tile([128, 2, 256], F32, tag="y", bufs=2)
                    for i in range(2):
                        nt = 2 * n2 + i
                        for cc2 in range(4):
                            nc.tensor.matmul(
                                y[:, i],
                                lhsT=ot_l[b][:, cc2, nt * 128:(nt + 1) * 128],
                                rhs=wp_sb[:, cc2],
                                start=(cc2 == 0), stop=(cc2 == 3),
                            )
                    nc.vector.tensor_tensor(
                        out=o_sb[:, 2 * n2:2 * n2 + 2],
                        in0=y,
                        in1=bp_sb.rearrange("p (i c) -> p i c", i=2),
                        op=mybir.AluOpType.add,
                    )
                nc.sync.dma_start(
                    out=out[b].rearrange("(t p) c -> p t c", p=128), in_=o_sb
                )


def build_module():
    nc = bacc.Bacc(
        "TRN2",
        target_bir_lowering=False,
        debug=False,
        enable_asserts=False,
        num_devices=NCORES,
    )
    x_t = nc.dram_tensor("x", [NB, N, DIM], F32, kind="ExternalInput")
    wqk_t = nc.dram_tensor("wqk", [CC, 128, 4, 128], F16, kind="ExternalInput")
    wv_t = nc.dram_tensor("wv", [CC, 128, 512], F16, kind="ExternalInput")
    wp_t = nc.dram_tensor("wp", [4, 128, 256], F16, kind="ExternalInput")
    wbeta_t = nc.dram_tensor("wbeta", [CC, 128, H], F16, kind="ExternalInput")
    bp_t = nc.dram_tensor("bp", [512], F32, kind="ExternalInput")
    e_t = nc.dram_tensor("etab", [4, MC, 2, 128, N], F16, kind="ExternalInput")
    out_t = nc.dram_tensor("out", [NB, N, DIM], F32, kind="ExternalOutput")

    aps = [t.ap() for t in (x_t, wqk_t, wv_t, wp_t, wbeta_t, bp_t, e_t, out_t)]
    with tile.TileContext(nc) as tc:
        _emit(tc, aps)
    nc.compile()
    return nc


def prep_inputs(inputs):
    """Host-side prep: fold norm affine + scale into weights, pack q/k rows
    into 32-row strips for PE row-tiling, fold bv into the proj bias, build
    the beta weight (exact bq fold) and materialize E = exp(bias)."""
    x = np.asarray(inputs["x"], np.float32)
    norm_w = np.asarray(inputs["norm_w"], np.float32)
    norm_b = np.asarray(inputs["norm_b"], np.float32)
    qkv_w = np.asarray(inputs["qkv_w"], np.float32)
    qkv_b = np.asarray(inputs["qkv_b"], np.float32)
    proj_w = np.asarray(inputs["proj_w"], np.float32)
    proj_b = np.asarray(inputs["proj_b"], np.float32)
    ab = np.asarray(inputs["attn_biases"], np.float32)
    bi = np.asarray(inputs["bias_idxs"], np.int64)

    scale = KD ** -0.5
    wr = qkv_w.reshape(H, 2 * KD + D, DIM)
    br = qkv_b.reshape(H, 2 * KD + D)
    # fold norm_w into weights, norm_b into biases
    w_eff = wr * norm_w[None, None, :]
    b_eff = br + wr @ norm_b
    w_q = w_eff[:, :KD] * scale
    b_q = b_eff[:, :KD]          # enters via beta only (softmax-exact fold)
    w_k = w_eff[:, KD:2 * KD]
    w_v = w_eff[:, 2 * KD:]
    b_v = b_eff[:, 2 * KD:]

    wqk = np.zeros((CC, 128, 4, 128), np.float16)
    for jt in range(4):
        kind_q = jt % 2 == 1
        hg = jt // 2
        w_src = w_q if kind_q else w_k
        for hp in range(4):
            h = hg * 4 + hp
            w_jc = w_src[h]  # [KD, DIM]
            for cc in range(CC):
                wqk[cc, :, jt, 32 * hp:32 * hp + KD] = (
                    w_jc[:, cc * 128:(cc + 1) * 128].T.astype(np.float16)
                )

    # w_beta[:, h] = scale * Wk_h^T @ bq_h  ->  beta[m,h] = scale*bq.k0(m)
    wbeta_full = np.zeros((DIM, H), np.float32)
    for h in range(H):
        wbeta_full[:, h] = scale * (w_k[h].T @ b_q[h])
    wbeta = np.zeros((CC, 128, H), np.float16)
    for cc in range(CC):
        wbeta[cc] = wbeta_full[cc * 128:(cc + 1) * 128].astype(np.float16)

    wv = np.zeros((CC, 128, 512), np.float16)
    for cc in range(CC):
        wv[cc] = w_v.reshape(512, DIM)[:, cc * 128:(cc + 1) * 128].T.astype(np.float16)

    wp = np.zeros((4, 128, 256), np.float16)
    for cc2 in range(4):
        wp[cc2] = proj_w[:, cc2 * 128:(cc2 + 1) * 128].T.astype(np.float16)
    # bv folded: out = (o/s + bv) @ Wp^T + bp = (o/s) @ Wp^T + (bp + Wp@bv)
    bp = np.tile((proj_b + proj_w @ b_v.reshape(512)).astype(np.float32), 2)

    etab = np.exp(ab[:, bi]).astype(np.float16)          # [H, 1024, 1024]
    etab = etab.reshape(4, 2, MC, 128, N).transpose(0, 2, 1, 3, 4).copy()

    shared = {
        "wqk": wqk, "wv": wv, "wp": wp, "wbeta": wbeta, "bp": bp, "etab": etab,
    }
    in_maps = []
    for c in range(NCORES):
        m = dict(shared)
        m["x"] = np.ascontiguousarray(x[c * NB:(c + 1) * NB])
        in_maps.append(m)
    return in_maps


_NC_CACHE = None


def _get_nc():
    global _NC_CACHE
    if _NC_CACHE is None:
        _NC_CACHE = build_module()
    return _NC_CACHE


def run(inputs, **spmd_kwargs):
    nc = _get_nc()
    in_maps = prep_inputs(inputs)
    res = bass_utils.run_bass_kernel_spmd(
        nc, in_maps, core_ids=list(range(NCORES)), **spmd_kwargs
    )
    out = np.concatenate([res.results[c]["out"] for c in range(NCORES)], axis=0)
    return out.astype(np.float32), res


def kernel(**inputs):
    out, _ = run(inputs)
    return out


if __name__ == "__main__":
    print("building module...")
    nc = _get_nc()
    print("module built ok")


# revision 21
# speedup vs baseline: 7555.3939x; 1.1558x over previous
"""Fused sparse-attention kernel for TRN2, SPMD over 8 NeuronCores.

Sharding: data-parallel over batch (32 -> 4 per core). Per core, the full
block (LayerNorm -> fused qkv -> per-head attention with gathered relative
position bias -> proj) is computed on-chip; attention probabilities never
touch HBM.

v2 restructure vs v1:
- All q/k/v/proj biases handled EXACTLY at ~zero on-chip cost:
  * q-bias bq enters scores only through beta[m] = scale*bq.k[m] (the
    query-side terms are constant over the softmax axis and cancel); beta
    is produced by one tiny extra matmul (w_beta = scale*Wk^T bq) and folded
    into the per-partition bias operand of the exp ACTIVATE.
  * v-bias: p@(v+bv) = p@v + sums*bv, and the normalize divides by sums, so
    bv contributes exactly bv per head dim -> folded into proj bias on host
    (bp_eff = bp + Wp @ bv).
- Softmax denominators: 1/sums via reciprocal_approx_fast (single custom-DVE
  op) instead of the 4-8us iterative-divide reciprocal per row.
- Normalize: GPSIMD partition_broadcast of the reciprocal row, then one f16
  DVE multiply into ot (PSUM drained by a single DVE copy so o-accumulator
  banks recycle fast).
- PSUM: s tiles [128,1024] f32 (2 banks, bufs=2) + o accumulators
  [65,1024] f32 x2 = exactly 8 banks; S matmuls of the next tile overlap the
  exp of the current one.
- E-multiply (ps *= exp(bias-table)) split between DVE and GPSIMD to
  balance engine load (GP_MOD knob).
"""

import os
import sys

import numpy as np

for _p in ("/opt/trn_rl_repo", "/root/.axon_site/_ro/trn_rl_repo"):
    if os.path.isdir(_p) and _p not in sys.path:
        sys.path.insert(0, _p)

import concourse.bacc as bacc
import concourse.tile as tile
from concourse import bass_utils, mybir
from concourse.masks import make_identity

F32 = mybir.dt.float32
F16 = mybir.dt.float16

NCORES = 8
B_TOTAL = 32
NB = B_TOTAL // NCORES  # local batch per core
N = 1024
NT = 8        # 128-row tiles over n
DIM = 256
CC = 2        # 128-row chunks over DIM
H = 8
KD = 16
D = 64
MC = 8        # 128-row chunks over m
EPS = 1e-5
OFF = float(4.0 * np.log(2.0))  # exp offset for fp16 headroom (cancels)

# E-multiply engine split: tile idx t (0..255); t % GP_MOD < GP_CNT -> GPSIMD
GP_MOD = 10
GP_CNT = 0


def _emit(tc, aps):
    nc = tc.nc
    x, wqk, wv, wp, wbeta, bp, etab, out = aps

    with tc.tile_pool(name="persist", bufs=1) as persist:
        # --- constants / weights resident in SBUF ---
        wqk_sb = persist.tile([128, CC, 4, 128], F16)
        nc.sync.dma_start(out=wqk_sb, in_=wqk.rearrange("cc ci jt j -> ci cc jt j"))
        wv_sb = persist.tile([128, CC, 512], F16)
        nc.sync.dma_start(out=wv_sb, in_=wv.rearrange("cc ci v -> ci cc v"))
        wp_sb = persist.tile([128, 4, 256], F16)
        nc.sync.dma_start(out=wp_sb, in_=wp.rearrange("cc ci c -> ci cc c"))
        wbeta_sb = persist.tile([128, CC, H], F16)
        nc.sync.dma_start(out=wbeta_sb, in_=wbeta.rearrange("cc ci h -> ci cc h"))
        bp_sb = persist.tile([128, 512], F32)
        nc.sync.dma_start(out=bp_sb, in_=bp.partition_broadcast(128))
        ident = persist.tile([128, 128], F16)
        make_identity(nc, ident)
        epsv = persist.tile([128, 1], F32)
        nc.vector.memset(epsv, EPS)

        qkT_l = []  # per-b [128, 4, 1024] f16: jt tiles (kT g0, qT g0, kT g1, qT g1)
        v_l = []    # per-b [128, NT, H, 65] f16: V rows + ones column per head
        ot_l = []   # per-b [128, 4, 1024] f16: O.T (dh on partitions, 4 chunks)
        boff_l = []  # per-b [128, MC, H] f32: beta[m,h] - OFF (exp bias operand)

        # ---------------- phase 1: LN, xn.T, qkv projections ----------------
        with (
            tc.tile_pool(name="p1", bufs=2) as p1,
            tc.tile_pool(name="p1ps", bufs=2, space="PSUM") as p1ps,
        ):
            for b in range(NB):
                x_sb = p1.tile([128, NT, DIM], F32, tag="x", bufs=2)
                nc.sync.dma_start(
                    out=x_sb, in_=x[b].rearrange("(t p) c -> p t c", p=128)
                )
                xn_sb = p1.tile([128, NT, DIM], F16, tag="xn", bufs=2)
                for t in range(NT):
                    stats = p1.tile([128, 6], F32, tag="stats", bufs=3)
                    nc.vector.bn_stats(out=stats, in_=x_sb[:, t])
                    mv = p1.tile([128, 2], F32, tag="mv", bufs=3)
                    nc.vector.bn_aggr(out=mv, in_=stats)
                    rstd = p1.tile([128, 1], F32, tag="rstd", bufs=3)
                    nc.scalar.activation(
                        out=rstd, in_=mv[:, 1:2],
                        func=mybir.ActivationFunctionType.Sqrt,
                        bias=epsv, scale=1.0,
                    )
                    nc.vector.reciprocal(out=rstd, in_=rstd)
                    nc.vector.tensor_scalar(
                        out=xn_sb[:, t], in0=x_sb[:, t],
                        scalar1=mv[:, 0:1], scalar2=rstd,
                        op0=mybir.AluOpType.subtract, op1=mybir.AluOpType.mult,
                    )
                # xn.T via PE transpose (copies batched per 2 tiles)
                xnT = p1.tile([128, CC, N], F16, tag="xnt", bufs=2)
                for cc in range(CC):
                    for t2 in range(NT // 2):
                        tp = p1ps.tile([128, 2, 128], F16, tag="tp", bufs=2)
                        for i in range(2):
                            t = 2 * t2 + i
                            nc.tensor.transpose(
                                tp[:, i], xn_sb[:, t, cc * 128:(cc + 1) * 128],
                                ident,
                            )
                        nc.scalar.copy(
                            out=xnT[:, cc, t2 * 256:(t2 + 1) * 256],
                            in_=tp,
                        )
                # q.T / k.T, packed by 32-row strips per head (zeros padding)
                qkT = persist.tile([128, 4, N], F16, tag="qkT", bufs=NB, name="qkT")
                for jt in range(4):
                    qkp = p1ps.tile([128, N], F32, tag="qkp", bufs=1)
                    for nh in range(2):
                        for cc in range(CC):
                            nc.tensor.matmul(
                                qkp[:, nh * 512:(nh + 1) * 512],
                                lhsT=wqk_sb[:, cc, jt],
                                rhs=xnT[:, cc, nh * 512:(nh + 1) * 512],
                                start=(cc == 0), stop=(cc == CC - 1),
                            )
                    nc.scalar.copy(out=qkT[:, jt], in_=qkp)
                qkT_l.append(qkT)
                # beta[m, h] = scale * bq_h . k(m)  (exp bias; exact bq fold)
                boff = persist.tile([128, MC, H], F32, tag="boff", bufs=NB,
                                    name="boff")
                bps = p1ps.tile([128, NT, H], F32, tag="bps", bufs=1)
                for t in range(NT):
                    for cc in range(CC):
                        nc.tensor.matmul(
                            bps[:, t],
                            lhsT=xnT[:, cc, t * 128:(t + 1) * 128],
                            rhs=wbeta_sb[:, cc],
                            start=(cc == 0), stop=(cc == CC - 1),
                        )
                nc.vector.tensor_scalar(
                    out=boff, in0=bps,
                    scalar1=-OFF, scalar2=None,
                    op0=mybir.AluOpType.add,
                )
                boff_l.append(boff)
                # V rows (no bias; bv folded into bp_eff) + ones column
                v_sb = persist.tile([128, NT, H, 65], F16, tag="v", bufs=NB,
                                    name="v_sb")
                nc.vector.memset(v_sb[:, :, :, 64:65], 1.0)
                for t2 in range(NT // 2):
                    vp = p1ps.tile([128, 2, 512], F32, tag="vp", bufs=1)
                    for i in range(2):
                        t = 2 * t2 + i
                        for cc in range(CC):
                            nc.tensor.matmul(
                                vp[:, i],
                                lhsT=xnT[:, cc, t * 128:(t + 1) * 128],
                                rhs=wv_sb[:, cc],
                                start=(cc == 0), stop=(cc == CC - 1),
                            )
                    nc.vector.tensor_copy(
                        out=v_sb[:, 2 * t2:2 * t2 + 2, :, 0:64],
                        in_=vp.rearrange("p i (h d) -> p i h d", d=64),
                    )
                v_l.append(v_sb)
                ot = persist.tile([128, 4, N], F16, tag="ot", bufs=NB, name="ot")
                ot_l.append(ot)

        # ---------------- phase 2: attention per head pair ----------------
        tile_idx = 0
        with (
            tc.tile_pool(name="p2", bufs=2) as p2,
            tc.tile_pool(name="p2ps", bufs=2, space="PSUM") as p2ps,
        ):
            for g in range(4):  # head pair {2g, 2g+1}
                e_sb = p2.tile([128, MC, 2, N], F16, tag="etab", bufs=2)
                nc.sync.dma_start(out=e_sb, in_=etab[g].rearrange(
                    "mc hp p n -> p mc hp n"))
                for b in range(NB):
                    # O'.T accumulators: [65, n] = V'.T @ P.T; row 64 = sums
                    o_ts = [
                        p2ps.tile([65, N], F32, tag="o", bufs=2, name="o_ts")
                        for _ in range(2)
                    ]

                    def emit_pv(mc, ps_pair):
                        # PV: V' (with ones column) stationary, P.T streams;
                        # lagged one mc step behind S so the PE queue always
                        # has ready S matmuls ahead of the exp-dependent PVs
                        for hp in range(2):
                            for nh in range(2):
                                nc.tensor.matmul(
                                    o_ts[hp][:, nh * 512:(nh + 1) * 512],
                                    lhsT=v_l[b][:, mc, 2 * g + hp],
                                    rhs=ps_pair[:, hp, nh * 512:(nh + 1) * 512],
                                    start=(mc == 0), stop=(mc == MC - 1),
                                    skip_group_check=True,
                                )

                    prev = None  # (mc, ps_pair)
                    for mc in range(MC):
                        s_tiles = [
                            p2ps.tile([128, N], F32, tag="s", bufs=2, name="s_ps")
                            for _ in range(2)
                        ]
                        # S matmuls, strip-alternated so the two heads' row
                        # tiles execute concurrently in the PE array
                        for nh in range(2):
                            for hp in range(2):
                                h = 2 * g + hp
                                jt = 2 * (h // 4)
                                strip = 32 * (h % 4)
                                nc.tensor.matmul(
                                    s_tiles[hp][:, nh * 512:(nh + 1) * 512],
                                    lhsT=qkT_l[b][strip:strip + KD, jt,
                                                  mc * 128:(mc + 1) * 128],
                                    rhs=qkT_l[b][strip:strip + KD, jt + 1,
                                                 nh * 512:(nh + 1) * 512],
                                    start=True, stop=True,
                                    tile_position=(strip, 0),
                                )
                        ps_pair = p2.tile([128, 2, N], F16, tag="ps", bufs=3,
                                          name="ps")
                        for hp in range(2):
                            h = 2 * g + hp
                            nc.scalar.activation(
                                out=ps_pair[:, hp], in_=s_tiles[hp],
                                func=mybir.ActivationFunctionType.Exp,
                                bias=boff_l[b][:, mc, h:h + 1], scale=1.0,
                            )
                        nc.vector.tensor_tensor(
                            out=ps_pair, in0=ps_pair, in1=e_sb[:, mc],
                            op=mybir.AluOpType.mult,
                        )
                        if prev is not None:
                            emit_pv(*prev)
                        prev = (mc, ps_pair)
                    emit_pv(*prev)
                    # tail: recip of sums, drain, broadcast, normalize
                    for hp in range(2):
                        srow = p2.tile([1, N], F32, tag="srow", bufs=1)
                        nc.vector.tensor_copy(out=srow, in_=o_ts[hp][64:65, :])
                        sr = p2.tile([1, N], F32, tag="sr", bufs=2)
                        nc.vector.reciprocal_approx_fast(out=sr, in_=srow)
                        raw = p2.tile([64, N], F16, tag="raw", bufs=2)
                        nc.vector.tensor_copy(out=raw, in_=o_ts[hp][0:64, :])
                        srf = p2.tile([1, N], F16, tag="srf", bufs=2)
                        nc.vector.tensor_copy(out=srf, in_=sr)
                        rb = p2.tile([64, N], F16, tag="rb", bufs=2)
                        nc.gpsimd.partition_broadcast(rb, srf)
                        nc.vector.tensor_tensor(
                            out=ot_l[b][64 * hp:64 * hp + 64, g, :],
                            in0=raw, in1=rb,
                            op=mybir.AluOpType.mult,
                        )

        # ---------------- phase 3: output projection ----------------
        with (
            tc.tile_pool(name="p3", bufs=2) as p3,
            tc.tile_pool(name="p3ps", bufs=4, space="PSUM") as p3ps,
        ):
            for b in range(NB):
                o_sb = p3.tile([128, NT, 256], F32, tag="osb", bufs=2)
                for n2 in range(NT // 2):
                    y = p3ps.tile([128, 2, 256], F32, tag="y", bufs=2)
                    for i in range(2):
                        nt = 2 * n2 + i
                        for cc2 in range(4):
                            nc.tensor.matmul(
                                y[:, i],
                                lhsT=ot_l[b][:, cc2, nt * 128:(nt + 1) * 128],
                                rhs=wp_sb[:, cc2],
                                start=(cc2 == 0), stop=(cc2 == 3),
                            )
                    nc.vector.tensor_tensor(
                        out=o_sb[:, 2 * n2:2 * n2 + 2],
                        in0=y,
                        in1=bp_sb.rearrange("p (i c) -> p i c", i=2),
                        op=mybir.AluOpType.add,
                    )
                nc.sync.dma_start(
                    out=out[b].rearrange("(t p) c -> p t c", p=128), in_=o_sb
                )


def build_module():
    nc = bacc.Bacc(
        "TRN2",
        target_bir_lowering=False,
        debug=False,
        enable_asserts=False,
        num_devices=NCORES,
    )
    x_t = nc.dram_tensor("x", [NB, N, DIM], F32, kind="ExternalInput")
    wqk_t = nc.dram_tensor("wqk", [CC, 128, 4, 128], F16, kind="ExternalInput")
    wv_t = nc.dram_tensor("wv", [CC, 128, 512], F16, kind="ExternalInput")
    wp_t = nc.dram_tensor("wp", [4, 128, 256], F16, kind="ExternalInput")
    wbeta_t = nc.dram_tensor("wbeta", [CC, 128, H], F16, kind="ExternalInput")
    bp_t = nc.dram_tensor("bp", [512], F32, kind="ExternalInput")
    e_t = nc.dram_tensor("etab", [4, MC, 2, 128, N], F16, kind="ExternalInput")
    out_t = nc.dram_tensor("out", [NB, N, DIM], F32, kind="ExternalOutput")

    aps = [t.ap() for t in (x_t, wqk_t, wv_t, wp_t, wbeta_t, bp_t, e_t, out_t)]
    with tile.TileContext(nc) as tc:
        _emit(tc, aps)
    nc.compile()
    return nc


def prep_inputs(inputs):
    """Host-side prep: fold norm affine + scale into weights, pack q/k rows
    into 32-row strips for PE row-tiling, fold bv into the proj bias, build
    the beta weight (exact bq fold) and materialize E = exp(bias)."""
    x = np.asarray(inputs["x"], np.float32)
    norm_w = np.asarray(inputs["norm_w"], np.float32)
    norm_b = np.asarray(inputs["norm_b"], np.float32)
    qkv_w = np.asarray(inputs["qkv_w"], np.float32)
    qkv_b = np.asarray(inputs["qkv_b"], np.float32)
    proj_w = np.asarray(inputs["proj_w"], np.float32)
    proj_b = np.asarray(inputs["proj_b"], np.float32)
    ab = np.asarray(inputs["attn_biases"], np.float32)
    bi = np.asarray(inputs["bias_idxs"], np.int64)

    scale = KD ** -0.5
    wr = qkv_w.reshape(H, 2 * KD + D, DIM)
    br = qkv_b.reshape(H, 2 * KD + D)
    # fold norm_w into weights, norm_b into biases
    w_eff = wr * norm_w[None, None, :]
    b_eff = br + wr @ norm_b
    w_q = w_eff[:, :KD] * scale
    b_q = b_eff[:, :KD]          # enters via beta only (softmax-exact fold)
    w_k = w_eff[:, KD:2 * KD]
    w_v = w_eff[:, 2 * KD:]
    b_v = b_eff[:, 2 * KD:]

    wqk = np.zeros((CC, 128, 4, 128), np.float16)
    for jt in range(4):
        kind_q = jt % 2 == 1
        hg = jt // 2
        w_src = w_q if kind_q else w_k
        for hp in range(4):
            h = hg * 4 + hp
            w_jc = w_src[h]  # [KD, DIM]
            for cc in range(CC):
                wqk[cc, :, jt, 32 * hp:32 * hp + KD] = (
                    w_jc[:, cc * 128:(cc + 1) * 128].T.astype(np.float16)
                )

    # w_beta[:, h] = scale * Wk_h^T @ bq_h  ->  beta[m,h] = scale*bq.k0(m)
    wbeta_full = np.zeros((DIM, H), np.float32)
    for h in range(H):
        wbeta_full[:, h] = scale * (w_k[h].T @ b_q[h])
    wbeta = np.zeros((CC, 128, H), np.float16)
    for cc in range(CC):
        wbeta[cc] = wbeta_full[cc * 128:(cc + 1) * 128].astype(np.float16)

    wv = np.zeros((CC, 128, 512), np.float16)
    for cc in range(CC):
        wv[cc] = w_v.reshape(512, DIM)[:, cc * 128:(cc + 1) * 128].T.astype(np.float16)

    wp = np.zeros((4, 128, 256), np.float16)
    for cc2 in range(4):
        wp[cc2] = proj_w[:, cc2 * 128:(cc2 + 1) * 128].T.astype(np.float16)
    # bv folded: out = (o/s + bv) @ Wp^T + bp = (o/s) @ Wp^T + (bp + Wp@bv)
    bp = np.tile((proj_b + proj_w @ b_v.reshape(512)).astype(np.float32), 2)

    etab = np.exp(ab[:, bi]).astype(np.float16)          # [H, 1024, 1024]
    etab = etab.reshape(4, 2, MC, 128, N).transpose(0, 2, 1, 3, 4).copy()

    shared = {
        "wqk": wqk, "wv": wv, "wp": wp, "wbeta": wbeta, "bp": bp, "etab": etab,
    }
    in_maps = []
    for c in range(NCORES):
        m = dict(shared)
        m["x"] = np.ascontiguousarray(x[c * NB:(c + 1) * NB])
        in_maps.append(m)
    return in_maps


_NC_CACHE = None


def _get_nc():
    global _NC_CACHE
    if _NC_CACHE is None:
        _NC_CACHE = build_module()
    return _NC_CACHE


def run(inputs, **spmd_kwargs):
    nc = _get_nc()
    in_maps = prep_inputs(inputs)
    res = bass_utils.run_bass_kernel_spmd(
        nc, in_maps, core_ids=list(range(NCORES)), **spmd_kwargs
    )
    out = np.concatenate([res.results[c]["out"] for c in range(NCORES)], axis=0)
    return out.astype(np.float32), res


def kernel(**inputs):
    out, _ = run(inputs)
    return out


if __name__ == "__main__":
    print("building module...")
    nc = _get_nc()
    print("module built ok")
